# revision 15
# baseline (speedup 1.0000x reference)
"""Trainium2 Bass kernel for a nonstandard GRU (gates computed after state update).

Strategy: data-parallel over batch (64 samples -> 8 cores x 8 samples).
Per core, the T=512 sequential recurrence runs entirely from SBUF:
  - all matmul operands (weights, x-tiles, transposed state) are bf16:
    the PE streams moving data at 1 col/cycle vs 4 for fp32. PSUM
    accumulation stays fp32, as does all elementwise state math.
  - gate matmuls stream weights through 4 PE column-groups
    (stationary = h^T tiles [128,8], moving = W^T chunks [128,256])
  - gate outputs land "striped" in PSUM: chunk g at partitions [32g, 32g+8),
    so elementwise/activation ops see FD=256 on 104 partitions instead of
    FD=1024 on 8 partitions.
  - input projections (X @ Wx^T etc.) are folded into the recurrent matmul
    as 2 extra K-tiles (lhsT = x_t^T staged [128,16]); they only depend on
    X, so they are emitted *ahead* of the h-dependent k-tiles (software
    pipelining): the PE chews on step t+1's x-projection while the
    activation/vector/transpose chain for step t drains.
  - h' and h'*r are transposed back to lhsT layout via PE transpose.
"""

import os
import sys

sys.path.insert(0, "/opt/trn_rl_repo")

import numpy as np

import concourse.bass as bass
import concourse.mybir as mybir
import concourse.tile as tile
from concourse import bacc
from concourse.bass import ds
from concourse.masks import make_identity

F32 = mybir.dt.float32
BF16 = mybir.dt.bfloat16
AF = mybir.ActivationFunctionType
ALU = mybir.AluOpType

# problem dims (per core)
B = 8          # batch per core (64 / 8 cores)
T = 512
IN = 256
H = 1024
OUT = 256
KT = H // 128   # 8 k-tiles over hidden
KI = IN // 128  # 2 k-tiles over input
NG = 4          # psum column groups
CH = H // NG    # 256 output chunk per group
SP = 3 * 32 + B  # 104 striped partitions


def _ht_slice(ht_sb, kt):
    # lhsT tile kt of a transposed-state buffer [128, 2*SP]
    # layout: block m=kt%2 at cols [m*SP, (m+1)*SP), stripe g=kt//2 at +32g
    return ht_sb[:, (kt % 2) * SP + 32 * (kt // 2):(kt % 2) * SP + 32 * (kt // 2) + B]


def build(n_steps=T, use_bias=False, unroll=8, mm_dt=BF16, pipelined=True, dbg=()):
    nc = bacc.Bacc("TRN2", target_bir_lowering=False)

    X_d = nc.dram_tensor("X", [B, T, IN], F32, kind="ExternalInput")
    Wx_d = nc.dram_tensor("Wx", [H, IN], F32, kind="ExternalInput")
    Wh_d = nc.dram_tensor("Wh", [H, H], F32, kind="ExternalInput")
    Uz_d = nc.dram_tensor("Uz", [H, IN], F32, kind="ExternalInput")
    Vz_d = nc.dram_tensor("Vz", [H, H], F32, kind="ExternalInput")
    Ur_d = nc.dram_tensor("Ur", [H, IN], F32, kind="ExternalInput")
    Vr_d = nc.dram_tensor("Vr", [H, H], F32, kind="ExternalInput")
    Wo_d = nc.dram_tensor("Wo", [OUT, H], F32, kind="ExternalInput")
    if use_bias:
        bx_d = nc.dram_tensor("bx", [H], F32, kind="ExternalInput")
        bz_d = nc.dram_tensor("bz", [H], F32, kind="ExternalInput")
        br_d = nc.dram_tensor("br", [H], F32, kind="ExternalInput")
        bo_d = nc.dram_tensor("bo", [OUT], F32, kind="ExternalInput")
    Y_d = nc.dram_tensor("Y", [B, OUT], F32, kind="ExternalOutput")

    with tile.TileContext(nc) as tc:
        with tc.tile_pool(name="state", bufs=1) as st:
            # persistent SBUF tensors; matmul operands in mm_dt
            WT_h = st.tile([128, KT * H], mm_dt, tag="WT_h")
            # r|z fused: col(kt, n) = kt*2H + (n//CH)*2CH + off + n%CH, off: r=0, z=CH
            WT_rz = st.tile([128, KT * 2 * H], mm_dt, tag="WT_rz")
            UT_h = st.tile([128, KI * H], mm_dt, tag="UT_h")
            UT_rz = st.tile([128, KI * 2 * H], mm_dt, tag="UT_rz")
            WoT = st.tile([128, KT * OUT], mm_dt, tag="WoT")
            # one trailing dummy step: the pipelined loop prefetches t+1
            XT = st.tile([128, T + 1, 2 * B], mm_dt, tag="XT")
            ident = st.tile([128, 128], F32, tag="ident")
            ones8 = st.tile([1, B], mm_dt, tag="ones8")
            biasf = st.tile([1, 3 * H + OUT], F32, tag="biasf")
            bias_sb = st.tile([1, 3 * H + OUT], mm_dt, tag="bias_sb")
            bias_rz = st.tile([1, 2 * H], mm_dt, tag="bias_rz")
            # striped state [SP(=104 used), 256], fp32
            hS = st.tile([128, CH], F32, tag="hS")
            zS = st.tile([128, CH], F32, tag="zS")
            rS = st.tile([128, CH], F32, tag="rS")
            htS = st.tile([128, CH], F32, tag="htS")
            zhS = st.tile([128, CH], F32, tag="zhS")
            omzS = st.tile([128, CH], F32, tag="omzS")
            mS = st.tile([128, CH], F32, tag="mS")
            hrS = st.tile([128, CH], F32, tag="hrS")
            # transposed state (matmul lhsT) in mm_dt
            hT_sb = st.tile([128, 2 * SP], mm_dt, tag="hT_sb")
            hrT_sb = st.tile([128, 2 * SP], mm_dt, tag="hrT_sb")
            ysb = st.tile([128, OUT], F32, tag="ysb")

            make_identity(nc, ident[:])
            nc.vector.memset(ones8[:], 1.0)
            for t_ in (hS, zS, rS, htS, zhS, omzS, mS, hrS, hT_sb, hrT_sb):
                nc.vector.memset(t_[:], 0.0)
            nc.vector.memset(XT[:, T, :], 0.0)
            if use_bias:
                nc.sync.dma_start(biasf[0, 0:H], bx_d[:])
                nc.sync.dma_start(biasf[0, H:2 * H], bz_d[:])
                nc.sync.dma_start(biasf[0, 2 * H:3 * H], br_d[:])
                nc.sync.dma_start(biasf[0, 3 * H:3 * H + OUT], bo_d[:])
                nc.vector.tensor_copy(bias_sb[:], biasf[:])
                for g in range(NG):
                    nc.vector.tensor_copy(
                        bias_rz[0:1, g * 2 * CH:g * 2 * CH + CH],
                        biasf[0:1, 2 * H + g * CH:2 * H + (g + 1) * CH])
                    nc.vector.tensor_copy(
                        bias_rz[0:1, g * 2 * CH + CH:(g + 1) * 2 * CH],
                        biasf[0:1, H + g * CH:H + (g + 1) * CH])
            else:
                nc.vector.memset(bias_sb[:], 0.0)
                nc.vector.memset(bias_rz[:], 0.0)

            # ---------- setup: load + transpose weights (cast to mm_dt) ----------
            with tc.tile_pool(name="setup_sb", bufs=3) as sb_pool, \
                 tc.tile_pool(name="setup_ps", bufs=4, space="PSUM") as ps_pool:

                def transpose_into(dst, src_d, R, C, colf=None):
                    # default: dst[p, ct*R + r] = src[r, ct*128 + p]
                    if colf is None:
                        colf = lambda ct, r0: ct * R + r0
                    for rt in range(R // 128):
                        nat = sb_pool.tile([128, C], F32, tag="nat")
                        nc.sync.dma_start(nat[:, :], src_d[rt * 128:(rt + 1) * 128, :])
                        for ct in range(C // 128):
                            pt = ps_pool.tile([128, 128], F32, tag="pt")
                            nc.tensor.transpose(pt[:], nat[:, ct * 128:(ct + 1) * 128], ident[:])
                            c0 = colf(ct, rt * 128)
                            nc.vector.tensor_copy(dst[:, c0:c0 + 128], pt[:])

                def rz_col(off, gw):
                    # interleave r|z chunks of CH within each (ct, group)
                    return lambda ct, r0: ct * gw + (r0 // CH) * 2 * CH + off + (r0 % CH)

                transpose_into(WT_h, Wh_d, H, H)
                transpose_into(WT_rz, Vr_d, H, H, colf=rz_col(0, 2 * H))
                transpose_into(WT_rz, Vz_d, H, H, colf=rz_col(CH, 2 * H))
                transpose_into(UT_h, Wx_d, H, IN)
                transpose_into(UT_rz, Ur_d, H, IN, colf=rz_col(0, 2 * H))
                transpose_into(UT_rz, Uz_d, H, IN, colf=rz_col(CH, 2 * H))
                transpose_into(WoT, Wo_d, OUT, H)

                # X -> XT[p, t, ki*8+b] = X[b, t, ki*128+p]
                for b in range(B):
                    for tt in range(T // 128):
                        nat = sb_pool.tile([128, IN], F32, tag="nat")
                        nc.sync.dma_start(nat[:, :], X_d[b, tt * 128:(tt + 1) * 128, :])
                        for ki in range(KI):
                            pt = ps_pool.tile([128, 128], F32, tag="pt")
                            nc.tensor.transpose(pt[:], nat[:, ki * 128:(ki + 1) * 128], ident[:])
                            nc.vector.tensor_copy(
                                XT[:, tt * 128:(tt + 1) * 128, ki * B + b], pt[:])

            # ---------- recurrence ----------
            with tc.tile_pool(name="xp", bufs=2) as xp, \
                 tc.tile_pool(name="ps", bufs=1, space="PSUM") as ps:

                def emit_x(pg, WT_x, brow, xst, cw=CH):
                    # x-projection k-tiles: open the accumulation group for pg.
                    # cw: output chunk width per column-group (CH or 2*CH for r|z)
                    gw_u = H * cw // CH
                    for ki in range(KI):
                        for g in range(NG):
                            nc.tensor.matmul(
                                pg[32 * g:32 * g + B, 0:cw],
                                lhsT=xst[:, 0, ki * B:(ki + 1) * B],
                                rhs=WT_x[:, ki * gw_u + g * cw:ki * gw_u + (g + 1) * cw],
                                start=(ki == 0), stop=False,
                                tile_position=(0, 32 * g))
                    if use_bias:
                        for g in range(NG):
                            nc.tensor.matmul(
                                pg[32 * g:32 * g + B, 0:cw],
                                lhsT=ones8[0:1, :],
                                rhs=brow[0:1, g * cw:(g + 1) * cw],
                                start=False, stop=False,
                                tile_position=(0, 32 * g))

                def emit_h(pg, lhs_of_kt, WT, cw=CH):
                    # h-dependent k-tiles: close the accumulation group.
                    gw_w = H * cw // CH
                    for kt in range(KT):
                        for g in range(NG):
                            nc.tensor.matmul(
                                pg[32 * g:32 * g + B, 0:cw],
                                lhsT=lhs_of_kt(kt),
                                rhs=WT[:, kt * gw_w + g * cw:kt * gw_w + (g + 1) * cw],
                                start=False, stop=(kt == KT - 1),
                                tile_position=(0, 32 * g))

                def transpose_pair(pg_tag, src, dst):
                    ptile = ps.tile([128, 2 * SP], F32, tag=pg_tag)
                    for m in (0, 1):
                        nc.tensor.transpose(
                            ptile[:, m * SP:(m + 1) * SP],
                            src[0:SP, m * 128:(m + 1) * 128],
                            ident[0:SP, 0:SP])
                    nc.vector.tensor_copy(dst[:], ptile[:])

                def fetch_x(t_sc, tag):
                    # every fetched tile is consumed within the same traced
                    # body: tile handles must NOT be carried across For_i
                    # iterations (the cross-iteration RAW on the recycled
                    # slot is not enforced and reads go stale).
                    xst = xp.tile([128, 1, 2 * B], mm_dt, tag=tag)
                    nc.vector.tensor_copy(xst[:], XT[:, ds(t_sc, 1), :])
                    return xst

                def step(t_sc, xst_cur, xst_nxt):
                    # off critical path: zh = z*h, omz = 1-z (previous z,h)
                    ew0 = nc.vector if "no_gpsimd" in dbg else nc.gpsimd
                    if "no_ew" not in dbg:
                        ew0.tensor_tensor(zhS[0:SP, :], zS[0:SP, :], hS[0:SP, :], ALU.mult)
                        ew0.tensor_scalar(omzS[0:SP, :], zS[0:SP, :], -1.0, 1.0, ALU.mult, ALU.add)
                    # G1 = hr @ Wh.T + x @ Wx.T (+ bx); x part already opened
                    pg1 = ps.tile([128, CH], F32, tag="pg1")
                    pg23 = ps.tile([128, 2 * CH], F32, tag="pg23")
                    if "no_mm" not in dbg:
                        if not pipelined:
                            emit_x(pg1, UT_h, bias_sb, xst_cur)
                        emit_h(pg1, lambda kt: _ht_slice(hrT_sb, kt), WT_h)
                        # G23 x-part next: PE works on it while tanh/ew/transpose run
                        emit_x(pg23, UT_rz, bias_rz, xst_cur, cw=2 * CH)
                    if "no_act" not in dbg:
                        nc.scalar.activation(htS[0:SP, :], pg1[0:SP, :], AF.Tanh)
                    # h' = zh + (1-z)*htilde
                    if "no_ew" not in dbg:
                        nc.vector.tensor_tensor(mS[0:SP, :], omzS[0:SP, :], htS[0:SP, :], ALU.mult)
                        nc.vector.tensor_tensor(hS[0:SP, :], zhS[0:SP, :], mS[0:SP, :], ALU.add)
                    if "no_tp" not in dbg:
                        transpose_pair("pt_h", hS, hT_sb)
                    # fused r|z gates: rhs = [Vr|Vz] interleaved per group, N=2*CH
                    if "no_mm" not in dbg:
                        emit_h(pg23, lambda kt: _ht_slice(hT_sb, kt), WT_rz, cw=2 * CH)
                        # next step's G1 x-part: PE works on it during sigmoid/hr
                        if pipelined:
                            emit_x(pg1, UT_h, bias_sb, xst_nxt)
                    if "no_act" not in dbg:
                        nc.scalar.activation(rS[0:SP, :], pg23[0:SP, 0:CH], AF.Sigmoid)
                        nc.scalar.activation(zS[0:SP, :], pg23[0:SP, CH:2 * CH], AF.Sigmoid)
                    # hr = h' * r ; transpose for next step
                    if "no_ew" not in dbg:
                        nc.vector.tensor_tensor(hrS[0:SP, :], hS[0:SP, :], rS[0:SP, :], ALU.mult)
                    if "no_tp" not in dbg:
                        transpose_pair("pt_hr", hrS, hrT_sb)

                # prologue: open G1(0) accumulation group with x_0
                if pipelined and "no_mm" not in dbg:
                    xb0 = fetch_x(0, "xb")
                    pg1 = ps.tile([128, CH], F32, tag="pg1")
                    emit_x(pg1, UT_h, bias_sb, xb0)

                def body(t_sc):
                    xa = fetch_x(t_sc, "xa")       # x_t for G23 (and G1 if nopipe)
                    xb = fetch_x(t_sc + 1, "xb") if pipelined else xa
                    step(t_sc, xa, xb)

                if n_steps % unroll == 0 and n_steps // unroll > 1:
                    with tc.For_i(0, n_steps // unroll, 1,
                                  hint_engines=tuple(mybir.ALL_ENGINES)) as it:
                        for u in range(unroll):
                            body(it * unroll + u)
                else:
                    for t in range(n_steps):
                        body(t)

                # flush the dangling G1(n_steps) group opened by the last step
                if pipelined and "no_mm" not in dbg:
                    pg1 = ps.tile([128, CH], F32, tag="pg1")
                    emit_h(pg1, lambda kt: _ht_slice(hrT_sb, kt), WT_h)

                # output: y = h @ Wo.T (+ bo)
                po = ps.tile([128, OUT], F32, tag="po")
                for kt in range(KT):
                    nc.tensor.matmul(
                        po[0:B, :], lhsT=_ht_slice(hT_sb, kt),
                        rhs=WoT[:, kt * OUT:(kt + 1) * OUT],
                        start=(kt == 0), stop=(kt == KT - 1 and not use_bias))
                if use_bias:
                    nc.tensor.matmul(
                        po[0:B, :], lhsT=ones8[0:1, :],
                        rhs=bias_sb[0:1, 3 * H:3 * H + OUT],
                        start=False, stop=True)
                nc.vector.tensor_copy(ysb[0:B, :], po[0:B, :])
                nc.sync.dma_start(Y_d[:, :], ysb[0:B, :])

    nc.compile()
    return nc


_CACHE = {}


def _get_nc(use_bias, n_steps=T, unroll=8):
    key = (use_bias, n_steps, unroll)
    if key not in _CACHE:
        _CACHE[key] = build(n_steps=n_steps, use_bias=use_bias, unroll=unroll)
    return _CACHE[key]


def kernel(**inputs):
    from concourse import bass_utils

    X = np.ascontiguousarray(inputs["X"], dtype=np.float32)
    n_cores = 8
    bt = X.shape[0] // n_cores
    use_bias = any(
        np.any(np.asarray(inputs[k]) != 0) for k in ("bx", "bz", "br", "bo") if k in inputs)
    nc = _get_nc(use_bias)

    weights = {k: np.ascontiguousarray(inputs[k], dtype=np.float32)
               for k in ("Wx", "Wh", "Uz", "Vz", "Ur", "Vr", "Wo")}
    if use_bias:
        for k in ("bx", "bz", "br", "bo"):
            weights[k] = np.ascontiguousarray(inputs[k], dtype=np.float32)

    in_maps = []
    for c in range(n_cores):
        m = dict(weights)
        m["X"] = np.ascontiguousarray(X[c * bt:(c + 1) * bt])
        in_maps.append(m)

    res = bass_utils.run_bass_kernel_spmd(nc, in_maps, core_ids=list(range(n_cores)))
    return np.concatenate([r["Y"] for r in res.results], axis=0)


if __name__ == "__main__":
    nc = build(n_steps=int(os.environ.get("STEPS", "16")), unroll=int(os.environ.get("UNROLL", "8")))
    print("build OK")


# revision 16
# speedup vs baseline: 1.0774x; 1.0774x over previous
"""Trainium2 Bass kernel for a nonstandard GRU (gates computed after state update).

Strategy: data-parallel over batch (64 samples -> 8 cores x 8 samples).
Per core, the T=512 sequential recurrence runs entirely from SBUF:
  - all matmul operands (weights, x-tiles, transposed state) are bf16:
    the PE streams moving data at 1 col/cycle vs 4 for fp32. PSUM
    accumulation stays fp32, as does all elementwise state math.
  - gate matmuls stream weights through 4 PE column-groups
    (stationary = h^T tiles [128,8], moving = W^T chunks [128,256])
  - gate outputs land "striped" in PSUM: chunk g at partitions [32g, 32g+8),
    so elementwise/activation ops see FD=256 on 104 partitions instead of
    FD=1024 on 8 partitions.
  - software pipelining: per step the PE order is
      H-gate h-tiles | R-x, Z-x (fills the tanh/h-update/transpose gap) |
      R h-tiles | Z h-tiles, next H-x (fills the sigmoid/hr/transpose gap)
    so the PE never waits on the activation/vector chain. The z gate is
    off the critical path (used only elementwise next step), so its
    weight stream is placed inside the r-chain gap.
  - input projections (x @ W^T) are folded into the recurrent matmuls as
    2 extra K-tiles; the H-gate x-part for step t+1 is emitted in step t.
  - h' and h'*r are transposed back to lhsT layout via PE transpose.
"""

import os
import sys

sys.path.insert(0, "/opt/trn_rl_repo")

import numpy as np

import concourse.bass as bass
import concourse.mybir as mybir
import concourse.tile as tile
from concourse import bacc
from concourse.bass import ds
from concourse.masks import make_identity

F32 = mybir.dt.float32
BF16 = mybir.dt.bfloat16
AF = mybir.ActivationFunctionType
ALU = mybir.AluOpType

# problem dims (per core)
B = 8          # batch per core (64 / 8 cores)
T = 512
IN = 256
H = 1024
OUT = 256
KT = H // 128   # 8 k-tiles over hidden
KI = IN // 128  # 2 k-tiles over input
NG = 4          # psum column groups
CH = H // NG    # 256 output chunk per group
SP = 3 * 32 + B  # 104 striped partitions


def _ht_slice(ht_sb, kt):
    # lhsT tile kt of a transposed-state buffer [128, 2*SP]
    # layout: block m=kt%2 at cols [m*SP, (m+1)*SP), stripe g=kt//2 at +32g
    return ht_sb[:, (kt % 2) * SP + 32 * (kt // 2):(kt % 2) * SP + 32 * (kt // 2) + B]


def build(n_steps=T, use_bias=False, unroll=8, mm_dt=BF16, pipelined=True, dbg=()):
    nc = bacc.Bacc("TRN2", target_bir_lowering=False)

    X_d = nc.dram_tensor("X", [B, T, IN], F32, kind="ExternalInput")
    Wx_d = nc.dram_tensor("Wx", [H, IN], F32, kind="ExternalInput")
    Wh_d = nc.dram_tensor("Wh", [H, H], F32, kind="ExternalInput")
    Uz_d = nc.dram_tensor("Uz", [H, IN], F32, kind="ExternalInput")
    Vz_d = nc.dram_tensor("Vz", [H, H], F32, kind="ExternalInput")
    Ur_d = nc.dram_tensor("Ur", [H, IN], F32, kind="ExternalInput")
    Vr_d = nc.dram_tensor("Vr", [H, H], F32, kind="ExternalInput")
    Wo_d = nc.dram_tensor("Wo", [OUT, H], F32, kind="ExternalInput")
    if use_bias:
        bx_d = nc.dram_tensor("bx", [H], F32, kind="ExternalInput")
        bz_d = nc.dram_tensor("bz", [H], F32, kind="ExternalInput")
        br_d = nc.dram_tensor("br", [H], F32, kind="ExternalInput")
        bo_d = nc.dram_tensor("bo", [OUT], F32, kind="ExternalInput")
    Y_d = nc.dram_tensor("Y", [B, OUT], F32, kind="ExternalOutput")

    with tile.TileContext(nc) as tc:
        with tc.tile_pool(name="state", bufs=1) as st:
            # persistent SBUF tensors; matmul operands in mm_dt
            WT_h = st.tile([128, KT * H], mm_dt, tag="WT_h")
            WT_r = st.tile([128, KT * H], mm_dt, tag="WT_r")
            WT_z = st.tile([128, KT * H], mm_dt, tag="WT_z")
            UT_h = st.tile([128, KI * H], mm_dt, tag="UT_h")
            UT_r = st.tile([128, KI * H], mm_dt, tag="UT_r")
            UT_z = st.tile([128, KI * H], mm_dt, tag="UT_z")
            WoT = st.tile([128, KT * OUT], mm_dt, tag="WoT")
            # one trailing dummy step: the pipelined loop prefetches t+1
            XT = st.tile([128, T + 1, 2 * B], mm_dt, tag="XT")
            ident = st.tile([128, 128], F32, tag="ident")
            ones8 = st.tile([1, B], mm_dt, tag="ones8")
            biasf = st.tile([1, 3 * H + OUT], F32, tag="biasf")
            bias_sb = st.tile([1, 3 * H + OUT], mm_dt, tag="bias_sb")
            # striped state [SP(=104 used), 256], fp32
            hS = st.tile([128, CH], F32, tag="hS")
            zS = st.tile([128, CH], F32, tag="zS")
            rS = st.tile([128, CH], F32, tag="rS")
            htS = st.tile([128, CH], F32, tag="htS")
            zhS = st.tile([128, CH], F32, tag="zhS")
            omzS = st.tile([128, CH], F32, tag="omzS")
            mS = st.tile([128, CH], F32, tag="mS")
            hrS = st.tile([128, CH], F32, tag="hrS")
            # transposed state (matmul lhsT) in mm_dt
            hT_sb = st.tile([128, 2 * SP], mm_dt, tag="hT_sb")
            hrT_sb = st.tile([128, 2 * SP], mm_dt, tag="hrT_sb")
            ysb = st.tile([128, OUT], F32, tag="ysb")

            make_identity(nc, ident[:])
            nc.vector.memset(ones8[:], 1.0)
            for t_ in (hS, zS, rS, htS, zhS, omzS, mS, hrS, hT_sb, hrT_sb):
                nc.vector.memset(t_[:], 0.0)
            nc.vector.memset(XT[:, T, :], 0.0)
            if use_bias:
                nc.sync.dma_start(biasf[0, 0:H], bx_d[:])
                nc.sync.dma_start(biasf[0, H:2 * H], bz_d[:])
                nc.sync.dma_start(biasf[0, 2 * H:3 * H], br_d[:])
                nc.sync.dma_start(biasf[0, 3 * H:3 * H + OUT], bo_d[:])
                nc.vector.tensor_copy(bias_sb[:], biasf[:])
            else:
                nc.vector.memset(bias_sb[:], 0.0)

            # ---------- setup: load + transpose weights (cast to mm_dt) ----------
            with tc.tile_pool(name="setup_sb", bufs=3) as sb_pool, \
                 tc.tile_pool(name="setup_ps", bufs=4, space="PSUM") as ps_pool:

                def transpose_into(dst, src_d, R, C):
                    # dst[p, ct*R + r] = src[r, ct*128 + p]
                    for rt in range(R // 128):
                        nat = sb_pool.tile([128, C], F32, tag="nat")
                        nc.sync.dma_start(nat[:, :], src_d[rt * 128:(rt + 1) * 128, :])
                        for ct in range(C // 128):
                            pt = ps_pool.tile([128, 128], F32, tag="pt")
                            nc.tensor.transpose(pt[:], nat[:, ct * 128:(ct + 1) * 128], ident[:])
                            c0 = ct * R + rt * 128
                            nc.vector.tensor_copy(dst[:, c0:c0 + 128], pt[:])

                transpose_into(WT_h, Wh_d, H, H)
                transpose_into(WT_r, Vr_d, H, H)
                transpose_into(WT_z, Vz_d, H, H)
                transpose_into(UT_h, Wx_d, H, IN)
                transpose_into(UT_r, Ur_d, H, IN)
                transpose_into(UT_z, Uz_d, H, IN)
                transpose_into(WoT, Wo_d, OUT, H)

                # X -> XT[p, t, ki*8+b] = X[b, t, ki*128+p]
                for b in range(B):
                    for tt in range(T // 128):
                        nat = sb_pool.tile([128, IN], F32, tag="nat")
                        nc.sync.dma_start(nat[:, :], X_d[b, tt * 128:(tt + 1) * 128, :])
                        for ki in range(KI):
                            pt = ps_pool.tile([128, 128], F32, tag="pt")
                            nc.tensor.transpose(pt[:], nat[:, ki * 128:(ki + 1) * 128], ident[:])
                            nc.vector.tensor_copy(
                                XT[:, tt * 128:(tt + 1) * 128, ki * B + b], pt[:])

            # ---------- recurrence ----------
            with tc.tile_pool(name="xp", bufs=2) as xp, \
                 tc.tile_pool(name="ps", bufs=1, space="PSUM") as ps:

                def emit_x(pg, WT_x, brow, xst):
                    # x-projection k-tiles: open the accumulation group for pg
                    # (start=True per column-group: has_written is per element).
                    for ki in range(KI):
                        for g in range(NG):
                            nc.tensor.matmul(
                                pg[32 * g:32 * g + B, 0:CH],
                                lhsT=xst[:, 0, ki * B:(ki + 1) * B],
                                rhs=WT_x[:, ki * H + g * CH:ki * H + (g + 1) * CH],
                                start=(ki == 0), stop=False,
                                tile_position=(0, 32 * g))
                    if use_bias:
                        for g in range(NG):
                            nc.tensor.matmul(
                                pg[32 * g:32 * g + B, 0:CH],
                                lhsT=ones8[0:1, :],
                                rhs=brow[0:1, g * CH:(g + 1) * CH],
                                start=False, stop=False,
                                tile_position=(0, 32 * g))

                def emit_h(pg, lhs_of_kt, WT):
                    # h-dependent k-tiles: close the accumulation group.
                    for kt in range(KT):
                        for g in range(NG):
                            nc.tensor.matmul(
                                pg[32 * g:32 * g + B, 0:CH],
                                lhsT=lhs_of_kt(kt),
                                rhs=WT[:, kt * H + g * CH:kt * H + (g + 1) * CH],
                                start=False, stop=(kt == KT - 1),
                                tile_position=(0, 32 * g))

                def transpose_pair(pg_tag, src, dst):
                    ptile = ps.tile([128, 2 * SP], F32, tag=pg_tag)
                    for m in (0, 1):
                        nc.tensor.transpose(
                            ptile[:, m * SP:(m + 1) * SP],
                            src[0:SP, m * 128:(m + 1) * 128],
                            ident[0:SP, 0:SP])
                    nc.vector.tensor_copy(dst[:], ptile[:])

                def fetch_x(t_sc, tag):
                    # every fetched tile is consumed within the same traced
                    # body: tile handles must NOT be carried across For_i
                    # iterations (the cross-iteration RAW on the recycled
                    # slot is not enforced and reads go stale).
                    xst = xp.tile([128, 1, 2 * B], mm_dt, tag=tag)
                    nc.vector.tensor_copy(xst[:], XT[:, ds(t_sc, 1), :])
                    return xst

                b_x = bias_sb[0:1, 0:H]
                b_z = bias_sb[0:1, H:2 * H]
                b_r = bias_sb[0:1, 2 * H:3 * H]

                def step(t_sc, xa, xb):
                    # off critical path: zh = z*h, omz = 1-z (previous z,h)
                    ew0 = nc.vector if "no_gpsimd" in dbg else nc.gpsimd
                    if "no_ew" not in dbg:
                        ew0.tensor_tensor(zhS[0:SP, :], zS[0:SP, :], hS[0:SP, :], ALU.mult)
                        ew0.tensor_scalar(omzS[0:SP, :], zS[0:SP, :], -1.0, 1.0, ALU.mult, ALU.add)
                    pgH = ps.tile([128, CH], F32, tag="pgH")
                    pgR = ps.tile([128, CH], F32, tag="pgR")
                    pgZ = ps.tile([128, CH], F32, tag="pgZ")
                    if "no_mm" not in dbg:
                        # close H(t): h~ = tanh(x@Wx + (h*r)@Wh)
                        if not pipelined:
                            emit_x(pgH, UT_h, b_x, xa)
                        emit_h(pgH, lambda kt: _ht_slice(hrT_sb, kt), WT_h)
                        # R/Z x-parts fill the tanh/h-update/transpose-h gap
                        emit_x(pgR, UT_r, b_r, xa)
                        emit_x(pgZ, UT_z, b_z, xa)
                    if "no_act" not in dbg:
                        nc.scalar.activation(htS[0:SP, :], pgH[0:SP, :], AF.Tanh)
                    # h' = zh + (1-z)*htilde
                    if "no_ew" not in dbg:
                        nc.vector.tensor_tensor(mS[0:SP, :], omzS[0:SP, :], htS[0:SP, :], ALU.mult)
                        nc.vector.tensor_tensor(hS[0:SP, :], zhS[0:SP, :], mS[0:SP, :], ALU.add)
                    if "no_tp" not in dbg:
                        transpose_pair("pt_h", hS, hT_sb)
                    if "no_mm" not in dbg:
                        # r first: it gates the next step's H matmul
                        emit_h(pgR, lambda kt: _ht_slice(hT_sb, kt), WT_r)
                        # z stream + next H x-part fill the sigmoid/hr/transpose gap
                        emit_h(pgZ, lambda kt: _ht_slice(hT_sb, kt), WT_z)
                        if pipelined:
                            emit_x(pgH, UT_h, b_x, xb)
                    if "no_act" not in dbg:
                        nc.scalar.activation(rS[0:SP, :], pgR[0:SP, :], AF.Sigmoid)
                    if "no_ew" not in dbg:
                        nc.vector.tensor_tensor(hrS[0:SP, :], hS[0:SP, :], rS[0:SP, :], ALU.mult)
                    if "no_tp" not in dbg:
                        transpose_pair("pt_hr", hrS, hrT_sb)
                    if "no_act" not in dbg:
                        nc.scalar.activation(zS[0:SP, :], pgZ[0:SP, :], AF.Sigmoid)

                # prologue: open H(0) accumulation group with x_0
                if pipelined and "no_mm" not in dbg:
                    xb0 = fetch_x(0, "xb")
                    pgH = ps.tile([128, CH], F32, tag="pgH")
                    emit_x(pgH, UT_h, b_x, xb0)

                def body(t_sc):
                    xa = fetch_x(t_sc, "xa")       # x_t for R/Z (and H if nopipe)
                    xb = fetch_x(t_sc + 1, "xb") if pipelined else xa
                    step(t_sc, xa, xb)

                if n_steps % unroll == 0 and n_steps // unroll > 1:
                    with tc.For_i(0, n_steps // unroll, 1,
                                  hint_engines=tuple(mybir.ALL_ENGINES)) as it:
                        for u in range(unroll):
                            body(it * unroll + u)
                else:
                    for t in range(n_steps):
                        body(t)

                # flush the dangling H(n_steps) group opened by the last step
                if pipelined and "no_mm" not in dbg:
                    pgH = ps.tile([128, CH], F32, tag="pgH")
                    emit_h(pgH, lambda kt: _ht_slice(hrT_sb, kt), WT_h)

                # output: y = h @ Wo.T (+ bo)
                po = ps.tile([128, OUT], F32, tag="po")
                for kt in range(KT):
                    nc.tensor.matmul(
                        po[0:B, :], lhsT=_ht_slice(hT_sb, kt),
                        rhs=WoT[:, kt * OUT:(kt + 1) * OUT],
                        start=(kt == 0), stop=(kt == KT - 1 and not use_bias))
                if use_bias:
                    nc.tensor.matmul(
                        po[0:B, :], lhsT=ones8[0:1, :],
                        rhs=bias_sb[0:1, 3 * H:3 * H + OUT],
                        start=False, stop=True)
                nc.vector.tensor_copy(ysb[0:B, :], po[0:B, :])
                nc.sync.dma_start(Y_d[:, :], ysb[0:B, :])

    nc.compile()
    return nc


_CACHE = {}


def _get_nc(use_bias, n_steps=T, unroll=8):
    key = (use_bias, n_steps, unroll)
    if key not in _CACHE:
        _CACHE[key] = build(n_steps=n_steps, use_bias=use_bias, unroll=unroll)
    return _CACHE[key]


def kernel(**inputs):
    from concourse import bass_utils

    X = np.ascontiguousarray(inputs["X"], dtype=np.float32)
    n_cores = 8
    bt = X.shape[0] // n_cores
    use_bias = any(
        np.any(np.asarray(inputs[k]) != 0) for k in ("bx", "bz", "br", "bo") if k in inputs)
    nc = _get_nc(use_bias)

    weights = {k: np.ascontiguousarray(inputs[k], dtype=np.float32)
               for k in ("Wx", "Wh", "Uz", "Vz", "Ur", "Vr", "Wo")}
    if use_bias:
        for k in ("bx", "bz", "br", "bo"):
            weights[k] = np.ascontiguousarray(inputs[k], dtype=np.float32)

    in_maps = []
    for c in range(n_cores):
        m = dict(weights)
        m["X"] = np.ascontiguousarray(X[c * bt:(c + 1) * bt])
        in_maps.append(m)

    res = bass_utils.run_bass_kernel_spmd(nc, in_maps, core_ids=list(range(n_cores)))
    return np.concatenate([r["Y"] for r in res.results], axis=0)


if __name__ == "__main__":
    nc = build(n_steps=int(os.environ.get("STEPS", "16")), unroll=int(os.environ.get("UNROLL", "8")))
    print("build OK")


# revision 26
# speedup vs baseline: 4.6309x; 4.2984x over previous
"""Trainium2 Bass kernel for a nonstandard GRU (gates computed after state update).

Strategy: data-parallel over batch (64 samples -> 8 cores x 8 samples).
Per core, the T=512 sequential recurrence runs entirely from SBUF.

All gate matmuls produce TRANSPOSED outputs: out[n, b] with the hidden dim
on PSUM partitions (8 n-tiles x 128) and batch (8) on the free axis.
Stationary operand = weight tile W^T[k, n] [128,128]; moving operand = the
transposed state h^T[k, b] [128,8] (the PE queue pulls LDWEIGHTS ahead of
in-flight matmuls, so the weight loads pipeline back-to-back).

The entire state (h, z, r, h~, ...) lives in this transposed layout
[128, kt, b] = [128, 8, 8]; as a result:
  - activations / elementwise ops are tiny [128, 64] tiles (vs [8,1024]
    natural or [104,256] striped),
  - the matmul lhsT for the next step IS the state - the per-step PE
    transposes of h and h*r disappear entirely,
  - x-projections fold into the gate matmuls as 2 extra K-tiles; the
    H-gate x-part for step t+1 is emitted during step t (software
    pipelining), so the PE works while the tanh/sigmoid chain drains.

Per-step PE order:
  H h-tiles (needs hr^T from t-1) | R-x, Z-x, H-x(t+1) gap fillers |
  R h-tiles (needs h^T) | Z h-tiles (z is off the critical path: it is
  used only elementwise in step t+1).

Everything the PE consumes is bf16 (PSUM accumulates fp32); state math is
fp32.
"""

import os
import sys

sys.path.insert(0, "/opt/trn_rl_repo")

import numpy as np

import concourse.bass as bass
import concourse.mybir as mybir
import concourse.tile as tile
from concourse import bacc
from concourse.bass import ds
from concourse.masks import make_identity

F32 = mybir.dt.float32
BF16 = mybir.dt.bfloat16
AF = mybir.ActivationFunctionType
ALU = mybir.AluOpType

# problem dims (per core)
B = 8          # batch per core (64 / 8 cores)
T = 512
IN = 256
H = 1024
OUT = 256
KT = H // 128   # 8 k-tiles (and n-tiles) over hidden
KI = IN // 128  # 2 k-tiles over input


def build(n_steps=T, use_bias=False, unroll=8, pipelined=True, dbg=()):
    nc = bacc.Bacc("TRN2", target_bir_lowering=False)

    X_d = nc.dram_tensor("X", [B, T, IN], F32, kind="ExternalInput")
    Wx_d = nc.dram_tensor("Wx", [H, IN], F32, kind="ExternalInput")
    Wh_d = nc.dram_tensor("Wh", [H, H], F32, kind="ExternalInput")
    Uz_d = nc.dram_tensor("Uz", [H, IN], F32, kind="ExternalInput")
    Vz_d = nc.dram_tensor("Vz", [H, H], F32, kind="ExternalInput")
    Ur_d = nc.dram_tensor("Ur", [H, IN], F32, kind="ExternalInput")
    Vr_d = nc.dram_tensor("Vr", [H, H], F32, kind="ExternalInput")
    Wo_d = nc.dram_tensor("Wo", [OUT, H], F32, kind="ExternalInput")
    if use_bias:
        bx_d = nc.dram_tensor("bx", [H], F32, kind="ExternalInput")
        bz_d = nc.dram_tensor("bz", [H], F32, kind="ExternalInput")
        br_d = nc.dram_tensor("br", [H], F32, kind="ExternalInput")
        bo_d = nc.dram_tensor("bo", [OUT], F32, kind="ExternalInput")
    Y_d = nc.dram_tensor("Y", [B, OUT], F32, kind="ExternalOutput")

    with tile.TileContext(nc) as tc:
        with tc.tile_pool(name="state", bufs=1) as st:
            # ---- persistent SBUF tensors ----
            # weights, transposed: WT[p, kt*H + n] = W[n, kt*128+p]
            WT_h = st.tile([128, KT * H], BF16, tag="WT_h")
            WT_r = st.tile([128, KT * H], BF16, tag="WT_r")
            WT_z = st.tile([128, KT * H], BF16, tag="WT_z")
            UT_h = st.tile([128, KI * H], BF16, tag="UT_h")
            UT_r = st.tile([128, KI * H], BF16, tag="UT_r")
            UT_z = st.tile([128, KI * H], BF16, tag="UT_z")
            WoT = st.tile([128, KT * OUT], BF16, tag="WoT")
            # x^T: XT[p, t, ki*8+b] = X[b, t, ki*128+p]  (+1 dummy step)
            XT = st.tile([128, T + 1, 2 * B], BF16, tag="XT")
            ident = st.tile([128, 128], F32, tag="ident")
            ones8 = st.tile([1, B], BF16, tag="ones8")
            biasf = st.tile([1, 3 * H + OUT], F32, tag="biasf")
            bias_sb = st.tile([1, 3 * H + OUT], BF16, tag="bias_sb")
            # transposed state [128, kt, b], fp32 master + bf16 matmul copies
            hT = st.tile([128, KT, B], F32, tag="hT")
            htT = st.tile([128, KT, B], F32, tag="htT")
            rT = st.tile([128, KT, B], F32, tag="rT")
            zT = st.tile([128, KT, B], F32, tag="zT")
            zhT = st.tile([128, KT, B], F32, tag="zhT")
            omzT = st.tile([128, KT, B], F32, tag="omzT")
            mT = st.tile([128, KT, B], F32, tag="mT")
            hTb = st.tile([128, KT, B], BF16, tag="hTb")    # bf16(h^T)
            hrTb = st.tile([128, KT, B], BF16, tag="hrTb")  # bf16((h*r)^T)
            ysb = st.tile([128, OUT], F32, tag="ysb")

            make_identity(nc, ident[:])
            nc.vector.memset(ones8[:], 1.0)
            for t_ in (hT, htT, rT, zT, zhT, omzT, mT, hTb, hrTb):
                nc.vector.memset(t_[:], 0.0)
            nc.vector.memset(XT[:, T, :], 0.0)
            if use_bias:
                nc.sync.dma_start(biasf[0, 0:H], bx_d[:])
                nc.sync.dma_start(biasf[0, H:2 * H], bz_d[:])
                nc.sync.dma_start(biasf[0, 2 * H:3 * H], br_d[:])
                nc.sync.dma_start(biasf[0, 3 * H:3 * H + OUT], bo_d[:])
                nc.vector.tensor_copy(bias_sb[:], biasf[:])
            else:
                nc.vector.memset(bias_sb[:], 0.0)

            # ---------- setup: load + transpose weights ----------
            with tc.tile_pool(name="setup_sb", bufs=3) as sb_pool, \
                 tc.tile_pool(name="setup_ps", bufs=4, space="PSUM") as ps_pool:

                def transpose_into(dst, src_d, R, C, stride):
                    # dst[p, ct*stride + r] = src[r, ct*128 + p]
                    for rt in range(R // 128):
                        nat = sb_pool.tile([128, C], F32, tag="nat")
                        nc.sync.dma_start(nat[:, :], src_d[rt * 128:(rt + 1) * 128, :])
                        for ct in range(C // 128):
                            pt = ps_pool.tile([128, 128], F32, tag="pt")
                            nc.tensor.transpose(pt[:], nat[:, ct * 128:(ct + 1) * 128], ident[:])
                            c0 = ct * stride + rt * 128
                            nc.vector.tensor_copy(dst[:, c0:c0 + 128], pt[:])

                transpose_into(WT_h, Wh_d, H, H, H)
                transpose_into(WT_r, Vr_d, H, H, H)
                transpose_into(WT_z, Vz_d, H, H, H)
                transpose_into(UT_h, Wx_d, H, IN, H)
                transpose_into(UT_r, Ur_d, H, IN, H)
                transpose_into(UT_z, Uz_d, H, IN, H)
                transpose_into(WoT, Wo_d, OUT, H, OUT)

                # X -> XT[p, t, ki*8+b] = X[b, t, ki*128+p]
                for b in range(B):
                    for tt in range(T // 128):
                        nat = sb_pool.tile([128, IN], F32, tag="nat")
                        nc.sync.dma_start(nat[:, :], X_d[b, tt * 128:(tt + 1) * 128, :])
                        for ki in range(KI):
                            pt = ps_pool.tile([128, 128], F32, tag="pt")
                            nc.tensor.transpose(pt[:], nat[:, ki * 128:(ki + 1) * 128], ident[:])
                            nc.vector.tensor_copy(
                                XT[:, tt * 128:(tt + 1) * 128, ki * B + b], pt[:])

            # ---------- recurrence ----------
            with tc.tile_pool(name="xp", bufs=2) as xp, \
                 tc.tile_pool(name="ps", bufs=1, space="PSUM") as ps:

                def emit_x(pg, UT_x, brow, xst):
                    # x-projection: out[:, nt, b] += UT_x_tile^T(n,k) @ x^T(k,b)
                    # opens the accumulation group for pg's psum bank.
                    first = True
                    for nt in range(KT):
                        for ki in range(KI):
                            nc.tensor.matmul(
                                pg[:, nt, :],
                                lhsT=UT_x[:, ki * H + nt * 128:ki * H + nt * 128 + 128],
                                rhs=xst[:, 0, ki * B:(ki + 1) * B],
                                start=first, stop=False)
                            first = False
                    if use_bias:
                        # out[n, b] += bias[n]: lhsT = bias row, rhs = ones
                        for nt in range(KT):
                            nc.tensor.matmul(
                                pg[:, nt, :],
                                lhsT=brow[0:1, nt * 128:(nt + 1) * 128],
                                rhs=ones8[0:1, :],
                                start=False, stop=False)

                def emit_h(pg, sTb, WT):
                    # h-dependent tiles: close the accumulation group.
                    for nt in range(KT):
                        for kt in range(KT):
                            nc.tensor.matmul(
                                pg[:, nt, :],
                                lhsT=WT[:, kt * H + nt * 128:kt * H + nt * 128 + 128],
                                rhs=sTb[:, kt, :],
                                start=False,
                                stop=(nt == KT - 1 and kt == KT - 1))

                def fetch_x(t_sc, tag):
                    # consumed within the same traced body: tile handles must
                    # NOT be carried across For_i iterations.
                    xst = xp.tile([128, 1, 2 * B], BF16, tag=tag, name=tag)
                    nc.vector.tensor_copy(xst[:], XT[:, ds(t_sc, 1), :])
                    return xst

                b_x = bias_sb[0:1, 0:H]
                b_z = bias_sb[0:1, H:2 * H]
                b_r = bias_sb[0:1, 2 * H:3 * H]

                def step(t_sc, xa, xb):
                    # off critical path: zh = z*h, omz = 1-z (previous z,h)
                    ew0 = nc.vector if "no_gpsimd" in dbg else nc.gpsimd
                    if "no_ew" not in dbg:
                        ew0.tensor_tensor(zhT[:], zT[:], hT[:], ALU.mult)
                        ew0.tensor_scalar(omzT[:], zT[:], -1.0, 1.0, ALU.mult, ALU.add)
                    pgH = ps.tile([128, KT, B], F32, tag="pgH")
                    pgR = ps.tile([128, KT, B], F32, tag="pgR")
                    pgZ = ps.tile([128, KT, B], F32, tag="pgZ")
                    if "no_mm" not in dbg:
                        # close H(t): h~^T = tanh(x@Wx^T + (h*r)@Wh^T)^T
                        if not pipelined:
                            emit_x(pgH, UT_h, b_x, xa)
                        emit_h(pgH, hrTb, WT_h)
                        # R/Z x-parts fill the tanh/update gap
                        emit_x(pgR, UT_r, b_r, xa)
                        emit_x(pgZ, UT_z, b_z, xa)
                    if "no_act" not in dbg:
                        nc.scalar.activation(htT[:], pgH[:, :, :], AF.Tanh)
                    # h' = zh + (1-z)*htilde ; bf16 copy for the matmuls
                    if "no_ew" not in dbg:
                        nc.vector.tensor_tensor(mT[:], omzT[:], htT[:], ALU.mult)
                        nc.vector.tensor_tensor(hT[:], zhT[:], mT[:], ALU.add)
                        nc.vector.tensor_copy(hTb[:], hT[:])
                    if "no_mm" not in dbg:
                        # r first: it gates the next step's H matmul
                        emit_h(pgR, hTb, WT_r)
                        emit_h(pgZ, hTb, WT_z)
                        # next step's H x-part (must be emitted after the
                        # tanh above: same-bank WAR ordering is positional)
                        if pipelined:
                            emit_x(pgH, UT_h, b_x, xb)
                    if "no_act" not in dbg:
                        nc.scalar.activation(rT[:], pgR[:, :, :], AF.Sigmoid)
                    if "no_ew" not in dbg:
                        nc.vector.tensor_tensor(hrTb[:], hT[:], rT[:], ALU.mult)
                    if "no_act" not in dbg:
                        nc.scalar.activation(zT[:], pgZ[:, :, :], AF.Sigmoid)

                # prologue: open H(0) accumulation group with x_0
                if pipelined and "no_mm" not in dbg:
                    xb0 = fetch_x(0, "xb")
                    pgH = ps.tile([128, KT, B], F32, tag="pgH")
                    emit_x(pgH, UT_h, b_x, xb0)

                def body(t_sc):
                    xa = fetch_x(t_sc, "xa")
                    xb = fetch_x(t_sc + 1, "xb") if pipelined else xa
                    step(t_sc, xa, xb)

                if n_steps % unroll == 0 and n_steps // unroll > 1:
                    with tc.For_i(0, n_steps // unroll, 1,
                                  hint_engines=tuple(mybir.ALL_ENGINES)) as it:
                        for u in range(unroll):
                            body(it * unroll + u)
                else:
                    for t in range(n_steps):
                        body(t)

                # flush the dangling H(n_steps) group opened by the last step
                if pipelined and "no_mm" not in dbg:
                    pgH = ps.tile([128, KT, B], F32, tag="pgH")
                    emit_h(pgH, hrTb, WT_h)

                # output: y = h @ Wo^T (+ bo); h^T is already the lhsT
                po = ps.tile([128, OUT], F32, tag="po")
                for kt in range(KT):
                    nc.tensor.matmul(
                        po[0:B, :], lhsT=hTb[:, kt, :],
                        rhs=WoT[:, kt * OUT:(kt + 1) * OUT],
                        start=(kt == 0), stop=(kt == KT - 1 and not use_bias))
                if use_bias:
                    nc.tensor.matmul(
                        po[0:B, :], lhsT=ones8[0:1, :],
                        rhs=bias_sb[0:1, 3 * H:3 * H + OUT],
                        start=False, stop=True)
                nc.vector.tensor_copy(ysb[0:B, :], po[0:B, :])
                nc.sync.dma_start(Y_d[:, :], ysb[0:B, :])

    nc.compile()
    return nc


_CACHE = {}


def _get_nc(use_bias, n_steps=T, unroll=8):
    key = (use_bias, n_steps, unroll)
    if key not in _CACHE:
        _CACHE[key] = build(n_steps=n_steps, use_bias=use_bias, unroll=unroll)
    return _CACHE[key]


def kernel(**inputs):
    from concourse import bass_utils

    X = np.ascontiguousarray(inputs["X"], dtype=np.float32)
    n_cores = 8
    bt = X.shape[0] // n_cores
    use_bias = any(
        np.any(np.asarray(inputs[k]) != 0) for k in ("bx", "bz", "br", "bo") if k in inputs)
    nc = _get_nc(use_bias)

    weights = {k: np.ascontiguousarray(inputs[k], dtype=np.float32)
               for k in ("Wx", "Wh", "Uz", "Vz", "Ur", "Vr", "Wo")}
    if use_bias:
        for k in ("bx", "bz", "br", "bo"):
            weights[k] = np.ascontiguousarray(inputs[k], dtype=np.float32)

    in_maps = []
    for c in range(n_cores):
        m = dict(weights)
        m["X"] = np.ascontiguousarray(X[c * bt:(c + 1) * bt])
        in_maps.append(m)

    res = bass_utils.run_bass_kernel_spmd(nc, in_maps, core_ids=list(range(n_cores)))
    return np.concatenate([r["Y"] for r in res.results], axis=0)


if __name__ == "__main__":
    nc = build(n_steps=int(os.environ.get("STEPS", "16")), unroll=int(os.environ.get("UNROLL", "8")))
    print("build OK")


# revision 31
# speedup vs baseline: 5.1325x; 1.1083x over previous
"""Trainium2 Bass kernel for a nonstandard GRU (gates computed after state update).

Strategy: data-parallel over batch (64 samples -> 8 cores x 8 samples).
Per core, the T=512 sequential recurrence runs entirely from SBUF.

All gate matmuls produce TRANSPOSED outputs: out[n, b] with the hidden dim
on PSUM partitions (8 n-tiles x 128) and batch (8) on the free axis.
Stationary operand = weight tile W^T[k, n] [128,128]; moving operand = the
transposed state h^T[k, b] [128,8] (the PE queue pulls LDWEIGHTS ahead of
in-flight matmuls, so the weight loads pipeline back-to-back).

The entire state (h, z, r, h~, ...) lives in this transposed layout
[128, kt, b] = [128, 8, 8]; as a result:
  - activations / elementwise ops are tiny [128, 64] tiles (vs [8,1024]
    natural or [104,256] striped),
  - the matmul lhsT for the next step IS the state - the per-step PE
    transposes of h and h*r disappear entirely,
  - x-projections fold into the gate matmuls as 2 extra K-tiles; the
    H-gate x-part for step t+1 is emitted during step t (software
    pipelining), so the PE works while the tanh/sigmoid chain drains.

Per-step PE order:
  H h-tiles (needs hr^T from t-1) | R-x, Z-x, H-x(t+1) gap fillers |
  R h-tiles (needs h^T) | Z h-tiles (z is off the critical path: it is
  used only elementwise in step t+1).

Everything the PE consumes is bf16 (PSUM accumulates fp32); state math is
fp32.
"""

import os
import sys

sys.path.insert(0, "/opt/trn_rl_repo")

import numpy as np

import concourse.bass as bass
import concourse.mybir as mybir
import concourse.tile as tile
from concourse import bacc
from concourse.bass import ds
from concourse.masks import make_identity

F32 = mybir.dt.float32
BF16 = mybir.dt.bfloat16
FP8 = mybir.dt.float8e4
DR = mybir.MatmulPerfMode.DoubleRow
WS = 256.0     # fp8 weight pre-scale (R/Z gate psums are x256)
AF = mybir.ActivationFunctionType
ALU = mybir.AluOpType

# problem dims (per core)
B = 8          # batch per core (64 / 8 cores)
T = 512
IN = 256
H = 1024
OUT = 256
KT = H // 128   # 8 k-tiles (and n-tiles) over hidden
KI = IN // 128  # 2 k-tiles over input


def build(n_steps=T, use_bias=False, unroll=8, pipelined=True, dbg=()):
    nc = bacc.Bacc("TRN2", target_bir_lowering=False)

    X_d = nc.dram_tensor("X", [B, T, IN], F32, kind="ExternalInput")
    Wx_d = nc.dram_tensor("Wx", [H, IN], F32, kind="ExternalInput")
    Wh_d = nc.dram_tensor("Wh", [H, H], F32, kind="ExternalInput")
    Uz_d = nc.dram_tensor("Uz", [H, IN], F32, kind="ExternalInput")
    Vz_d = nc.dram_tensor("Vz", [H, H], F32, kind="ExternalInput")
    Ur_d = nc.dram_tensor("Ur", [H, IN], F32, kind="ExternalInput")
    Vr_d = nc.dram_tensor("Vr", [H, H], F32, kind="ExternalInput")
    Wo_d = nc.dram_tensor("Wo", [OUT, H], F32, kind="ExternalInput")
    if use_bias:
        bx_d = nc.dram_tensor("bx", [H], F32, kind="ExternalInput")
        bz_d = nc.dram_tensor("bz", [H], F32, kind="ExternalInput")
        br_d = nc.dram_tensor("br", [H], F32, kind="ExternalInput")
        bo_d = nc.dram_tensor("bo", [OUT], F32, kind="ExternalInput")
    Y_d = nc.dram_tensor("Y", [B, OUT], F32, kind="ExternalOutput")

    with tile.TileContext(nc) as tc:
        with tc.tile_pool(name="state", bufs=1) as st:
            # ---- persistent SBUF tensors ----
            # weights, transposed: WT[p, kt*H + n] = W[n, kt*128+p]
            WT_h = st.tile([128, KT * H], BF16, tag="WT_h")
            W8_r = st.tile([128, KT // 2, 2, H], FP8, tag="W8_r")  # fp8(Vr*WS)
            W8_z = st.tile([128, KT // 2, 2, H], FP8, tag="W8_z")  # fp8(Vz*WS)
            UT_h = st.tile([128, KI * H], BF16, tag="UT_h")
            UT_r = st.tile([128, KI * H], BF16, tag="UT_r")
            UT_z = st.tile([128, KI * H], BF16, tag="UT_z")
            WoT = st.tile([128, KT * OUT], BF16, tag="WoT")
            # x^T: XT[p, t, ki*8+b] = X[b, t, ki*128+p]  (+1 dummy step)
            XT = st.tile([128, T + 1, 2 * B], BF16, tag="XT")
            ident = st.tile([128, 128], F32, tag="ident")
            ones8 = st.tile([1, B], BF16, tag="ones8")
            biasf = st.tile([1, 3 * H + OUT], F32, tag="biasf")
            bias_sb = st.tile([1, 3 * H + OUT], BF16, tag="bias_sb")
            # transposed state [128, kt, b]; h itself lives in bf16 (hTb)
            htT = st.tile([128, KT, B], F32, tag="htT")
            rT = st.tile([128, KT, B], F32, tag="rT")
            zT = st.tile([128, KT, B], F32, tag="zT")
            zhT = st.tile([128, KT, B], F32, tag="zhT")
            omzT = st.tile([128, KT, B], F32, tag="omzT")
            mT = st.tile([128, KT, B], F32, tag="mT")
            hTb = st.tile([128, KT, B], BF16, tag="hTb")    # h^T state
            hT8 = st.tile([128, KT, 32], FP8, tag="hT8")    # fp8 h^T, padded pairs
            hrTb = st.tile([128, KT, B], BF16, tag="hrTb")  # (h*r)^T
            ysb = st.tile([128, OUT], F32, tag="ysb")

            make_identity(nc, ident[:])
            nc.vector.memset(ones8[:], 1.0)
            for t_ in (htT, rT, zT, zhT, omzT, mT, hTb, hT8, hrTb):
                nc.vector.memset(t_[:], 0.0)
            nc.vector.memset(XT[:, T, :], 0.0)
            if use_bias:
                nc.sync.dma_start(biasf[0, 0:H], bx_d[:])
                nc.sync.dma_start(biasf[0, H:2 * H], bz_d[:])
                nc.sync.dma_start(biasf[0, 2 * H:3 * H], br_d[:])
                nc.sync.dma_start(biasf[0, 3 * H:3 * H + OUT], bo_d[:])
                nc.vector.tensor_copy(bias_sb[0:1, 0:H], biasf[0:1, 0:H])
                nc.vector.tensor_scalar_mul(
                    bias_sb[0:1, H:3 * H], biasf[0:1, H:3 * H], WS)
                nc.vector.tensor_copy(
                    bias_sb[0:1, 3 * H:3 * H + OUT], biasf[0:1, 3 * H:3 * H + OUT])
            else:
                nc.vector.memset(bias_sb[:], 0.0)

            # ---------- setup: load + transpose weights ----------
            with tc.tile_pool(name="setup_sb", bufs=3) as sb_pool, \
                 tc.tile_pool(name="setup_ps", bufs=4, space="PSUM") as ps_pool:

                def load_T(src_d, R, C, writer):
                    # writer(ct, r0, pt): pt = src[r0:r0+128, ct*128:+128].T
                    for rt in range(R // 128):
                        nat = sb_pool.tile([128, C], F32, tag="nat")
                        nc.sync.dma_start(nat[:, :], src_d[rt * 128:(rt + 1) * 128, :])
                        for ct in range(C // 128):
                            pt = ps_pool.tile([128, 128], F32, tag="pt")
                            nc.tensor.transpose(pt[:], nat[:, ct * 128:(ct + 1) * 128], ident[:])
                            writer(ct, rt * 128, pt)

                def w_bf16(dst, stride, scale=1.0):
                    def wr(ct, r0, pt):
                        d = dst[:, ct * stride + r0:ct * stride + r0 + 128]
                        if scale == 1.0:
                            nc.vector.tensor_copy(d, pt[:])
                        else:
                            nc.vector.tensor_scalar_mul(d, pt[:], scale)
                    return wr

                def w_fp8(dst):
                    def wr(ct, r0, pt):
                        nc.vector.tensor_scalar_mul(
                            dst[:, ct // 2, ct % 2, r0:r0 + 128], pt[:], WS)
                    return wr

                load_T(Wh_d, H, H, w_bf16(WT_h, H))
                load_T(Vr_d, H, H, w_fp8(W8_r))
                load_T(Vz_d, H, H, w_fp8(W8_z))
                load_T(Wx_d, H, IN, w_bf16(UT_h, H))
                load_T(Ur_d, H, IN, w_bf16(UT_r, H, WS))
                load_T(Uz_d, H, IN, w_bf16(UT_z, H, WS))
                load_T(Wo_d, OUT, H, w_bf16(WoT, OUT))

                # X -> XT[p, t, ki*8+b] = X[b, t, ki*128+p]
                for b in range(B):
                    for tt in range(T // 128):
                        nat = sb_pool.tile([128, IN], F32, tag="nat")
                        nc.sync.dma_start(nat[:, :], X_d[b, tt * 128:(tt + 1) * 128, :])
                        for ki in range(KI):
                            pt = ps_pool.tile([128, 128], F32, tag="pt")
                            nc.tensor.transpose(pt[:], nat[:, ki * 128:(ki + 1) * 128], ident[:])
                            nc.vector.tensor_copy(
                                XT[:, tt * 128:(tt + 1) * 128, ki * B + b], pt[:])

            # ---------- recurrence ----------
            with tc.tile_pool(name="xp", bufs=2) as xp, \
                 tc.tile_pool(name="ps", bufs=1, space="PSUM") as ps:

                def emit_x(pg, UT_x, brow, xst):
                    # x-projection: out[:, nt, b] += UT_x_tile^T(n,k) @ x^T(k,b)
                    # opens the accumulation group for pg's psum bank.
                    first = True
                    for nt in range(KT):
                        for ki in range(KI):
                            nc.tensor.matmul(
                                pg[:, nt, :],
                                lhsT=UT_x[:, ki * H + nt * 128:ki * H + nt * 128 + 128],
                                rhs=xst[:, 0, ki * B:(ki + 1) * B],
                                start=first, stop=False)
                            first = False
                    if use_bias:
                        # out[n, b] += bias[n]: lhsT = bias row, rhs = ones
                        for nt in range(KT):
                            nc.tensor.matmul(
                                pg[:, nt, :],
                                lhsT=brow[0:1, nt * 128:(nt + 1) * 128],
                                rhs=ones8[0:1, :],
                                start=False, stop=False)

                def emit_h(pg, sTb, WT):
                    # h-dependent tiles: close the accumulation group.
                    for nt in range(KT):
                        for kt in range(KT):
                            nc.tensor.matmul(
                                pg[:, nt, :],
                                lhsT=WT[:, kt * H + nt * 128:kt * H + nt * 128 + 128],
                                rhs=sTb[:, kt, :],
                                start=False,
                                stop=(nt == KT - 1 and kt == KT - 1))

                def emit_h_fp8(pg, W8):
                    # DoubleRow: stationary W8 tile [128, 2, 128] covers a
                    # kt-pair; moving = fp8 h^T pair [128, 2, 8].
                    for nt in range(KT):
                        for j in range(KT // 2):
                            nc.tensor.matmul(
                                pg[:, nt, :],
                                lhsT=W8[:, j, :, nt * 128:nt * 128 + 128],
                                rhs=hT8[:, 2 * j:2 * j + 2, 0:B],
                                start=False, perf_mode=DR,
                                stop=(nt == KT - 1 and j == KT // 2 - 1))

                def fetch_x(t_sc, tag):
                    # consumed within the same traced body: tile handles must
                    # NOT be carried across For_i iterations.
                    xst = xp.tile([128, 1, 2 * B], BF16, tag=tag, name=tag)
                    nc.vector.tensor_copy(xst[:], XT[:, ds(t_sc, 1), :])
                    return xst

                b_x = bias_sb[0:1, 0:H]
                b_z = bias_sb[0:1, H:2 * H]
                b_r = bias_sb[0:1, 2 * H:3 * H]

                def step(t_sc, xa, xb):
                    # off critical path: zh = z*h, omz = 1-z (previous z,h)
                    ew0 = nc.vector if "no_gpsimd" in dbg else nc.gpsimd
                    if "no_ew" not in dbg:
                        ew0.tensor_tensor(zhT[:], zT[:], hTb[:], ALU.mult)
                        ew0.tensor_scalar(omzT[:], zT[:], -1.0, 1.0, ALU.mult, ALU.add)
                    pgH = ps.tile([128, KT, B], F32, tag="pgH")
                    pgR = ps.tile([128, KT, B], F32, tag="pgR")
                    pgZ = ps.tile([128, KT, B], F32, tag="pgZ")
                    if "no_mm" not in dbg:
                        # close H(t): h~^T = tanh(x@Wx^T + (h*r)@Wh^T)^T
                        if not pipelined:
                            emit_x(pgH, UT_h, b_x, xa)
                        emit_h(pgH, hrTb, WT_h)
                        # R/Z x-parts fill the tanh/update gap
                        emit_x(pgR, UT_r, b_r, xa)
                        emit_x(pgZ, UT_z, b_z, xa)
                    if "no_act" not in dbg:
                        nc.scalar.activation(htT[:], pgH[:, :, :], AF.Tanh)
                    # h' = zh + (1-z)*htilde; fp8 copy (chain) + bf16 copy
                    if "no_ew" not in dbg:
                        nc.vector.tensor_tensor(mT[:], omzT[:], htT[:], ALU.mult)
                        nc.vector.tensor_tensor(hT8[:, :, 0:B], zhT[:], mT[:], ALU.add)
                        nc.vector.tensor_tensor(hTb[:], zhT[:], mT[:], ALU.add)
                    if "no_mm" not in dbg:
                        # r first: it gates the next step's H matmul
                        emit_h_fp8(pgR, W8_r)
                        emit_h_fp8(pgZ, W8_z)
                        # next step's H x-part (must be emitted after the
                        # tanh above: same-bank WAR ordering is positional)
                        if pipelined:
                            emit_x(pgH, UT_h, b_x, xb)
                    if "no_act" not in dbg:
                        nc.scalar.activation(rT[:], pgR[:, :, :], AF.Sigmoid, scale=1.0 / WS)
                    if "no_ew" not in dbg:
                        nc.vector.tensor_tensor(hrTb[:], hTb[:], rT[:], ALU.mult)
                    if "no_act" not in dbg:
                        nc.scalar.activation(zT[:], pgZ[:, :, :], AF.Sigmoid, scale=1.0 / WS)

                # prologue: open H(0) accumulation group with x_0
                if pipelined and "no_mm" not in dbg:
                    xb0 = fetch_x(0, "xb")
                    pgH = ps.tile([128, KT, B], F32, tag="pgH")
                    emit_x(pgH, UT_h, b_x, xb0)

                def body(t_sc):
                    xa = fetch_x(t_sc, "xa")
                    xb = fetch_x(t_sc + 1, "xb") if pipelined else xa
                    step(t_sc, xa, xb)

                if n_steps % unroll == 0 and n_steps // unroll > 1:
                    with tc.For_i(0, n_steps // unroll, 1,
                                  hint_engines=tuple(mybir.ALL_ENGINES)) as it:
                        for u in range(unroll):
                            body(it * unroll + u)
                else:
                    for t in range(n_steps):
                        body(t)

                # flush the dangling H(n_steps) group opened by the last step
                if pipelined and "no_mm" not in dbg:
                    pgH = ps.tile([128, KT, B], F32, tag="pgH")
                    emit_h(pgH, hrTb, WT_h)

                # output: y = h @ Wo^T (+ bo); h^T is already the lhsT
                po = ps.tile([128, OUT], F32, tag="po")
                for kt in range(KT):
                    nc.tensor.matmul(
                        po[0:B, :], lhsT=hTb[:, kt, :],
                        rhs=WoT[:, kt * OUT:(kt + 1) * OUT],
                        start=(kt == 0), stop=(kt == KT - 1 and not use_bias))
                if use_bias:
                    nc.tensor.matmul(
                        po[0:B, :], lhsT=ones8[0:1, :],
                        rhs=bias_sb[0:1, 3 * H:3 * H + OUT],
                        start=False, stop=True)
                nc.vector.tensor_copy(ysb[0:B, :], po[0:B, :])
                nc.sync.dma_start(Y_d[:, :], ysb[0:B, :])

    nc.compile()
    return nc


_CACHE = {}


def _get_nc(use_bias, n_steps=T, unroll=8):
    key = (use_bias, n_steps, unroll)
    if key not in _CACHE:
        _CACHE[key] = build(n_steps=n_steps, use_bias=use_bias, unroll=unroll)
    return _CACHE[key]


def kernel(**inputs):
    from concourse import bass_utils

    X = np.ascontiguousarray(inputs["X"], dtype=np.float32)
    n_cores = 8
    bt = X.shape[0] // n_cores
    use_bias = any(
        np.any(np.asarray(inputs[k]) != 0) for k in ("bx", "bz", "br", "bo") if k in inputs)
    nc = _get_nc(use_bias)

    weights = {k: np.ascontiguousarray(inputs[k], dtype=np.float32)
               for k in ("Wx", "Wh", "Uz", "Vz", "Ur", "Vr", "Wo")}
    if use_bias:
        for k in ("bx", "bz", "br", "bo"):
            weights[k] = np.ascontiguousarray(inputs[k], dtype=np.float32)

    in_maps = []
    for c in range(n_cores):
        m = dict(weights)
        m["X"] = np.ascontiguousarray(X[c * bt:(c + 1) * bt])
        in_maps.append(m)

    res = bass_utils.run_bass_kernel_spmd(nc, in_maps, core_ids=list(range(n_cores)))
    return np.concatenate([r["Y"] for r in res.results], axis=0)


if __name__ == "__main__":
    nc = build(n_steps=int(os.environ.get("STEPS", "16")), unroll=int(os.environ.get("UNROLL", "8")))
    print("build OK")
